# revision 4
# baseline (speedup 1.0000x reference)
"""GCN layer kernel for Trainium2 (8 NeuronCores).

Strategy (per sharding_hint): shard node rows across the 8 cores for the
dense projection Z = X @ W (the FLOP-heavy part) on the TensorEngines.
W [512,512] is replicated. To avoid on-device transposes, the host passes
X^T and the device computes OutT = W^T @ X^T; the host transposes back.
The irregular COO scatter-add (segment_sum over 800k random edges) is done
host-side as a CSR SpMM, followed by ReLU.
"""

import numpy as np

N_NODES = 50000
M_IN = 512
H_OUT = 512
N_CORES = 8
# per-core padded column count: 13 blocks of 512
COLS = 6656
PAD_NODES = COLS * N_CORES  # 53248

_compiled = {}


def _build_nc():
    from concourse import bacc, mybir
    from concourse import tile

    f32 = mybir.dt.float32
    bf16 = mybir.dt.bfloat16
    nc = bacc.Bacc(None, debug=False)

    xt = nc.declare_dram_parameter("xt", [M_IN, COLS], bf16, isOutput=False)
    w = nc.declare_dram_parameter("w", [M_IN, H_OUT], bf16, isOutput=False)
    outt = nc.declare_dram_parameter("out", [H_OUT, COLS], f32, isOutput=True)

    KC = M_IN // 128  # 4 contraction chunks
    NC_ = H_OUT // 128  # 4 output-row chunks
    NB = COLS // 512  # 13 column blocks

    with tile.TileContext(nc) as tc:
        with (
            tc.tile_pool(name="wpool", bufs=1) as wpool,
            tc.tile_pool(name="xpool", bufs=3) as xpool,
            tc.tile_pool(name="opool", bufs=4) as opool,
            tc.tile_pool(name="psum", bufs=4, space="PSUM") as pp,
        ):
            wt = wpool.tile([128, KC, H_OUT], bf16)
            for k in range(KC):
                nc.sync.dma_start(wt[:, k, :], w[k * 128 : (k + 1) * 128, :])

            for cb in range(NB):
                xtile = xpool.tile([128, KC, 512], bf16)
                for k in range(KC):
                    nc.sync.dma_start(
                        xtile[:, k, :],
                        xt[k * 128 : (k + 1) * 128, cb * 512 : (cb + 1) * 512],
                    )
                for n in range(NC_):
                    acc = pp.tile([128, 512], f32)
                    for k in range(KC):
                        nc.tensor.matmul(
                            acc[:],
                            wt[:, k, n * 128 : (n + 1) * 128],
                            xtile[:, k, :],
                            start=(k == 0),
                            stop=(k == KC - 1),
                        )
                    otile = opool.tile([128, 512], f32)
                    nc.vector.tensor_copy(otile[:], acc[:])
                    nc.sync.dma_start(
                        outt[n * 128 : (n + 1) * 128, cb * 512 : (cb + 1) * 512],
                        otile[:],
                    )
    nc.compile()
    return nc


def _get_nc():
    if "nc" not in _compiled:
        _compiled["nc"] = _build_nc()
    return _compiled["nc"]


def kernel(X, W, edge_src, edge_dst, edge_vals):
    import scipy.sparse as sp
    from concourse.bass_utils import run_bass_kernel_spmd

    X = np.asarray(X, dtype=np.float32)
    W = np.ascontiguousarray(np.asarray(W, dtype=np.float32))
    edge_src = np.asarray(edge_src)
    edge_dst = np.asarray(edge_dst)
    edge_vals = np.asarray(edge_vals, dtype=np.float32)

    import ml_dtypes

    # host pre-transpose + pad so the device needs no transposes; bf16 halves
    # the upload and uses the fast TensorEngine path (rel err ~2e-3)
    bf = ml_dtypes.bfloat16
    XT = np.zeros((M_IN, PAD_NODES), dtype=bf)
    XT[:, :N_NODES] = X.T.astype(bf)
    W = W.astype(bf)

    in_maps = [
        {"xt": np.ascontiguousarray(XT[:, i * COLS : (i + 1) * COLS]), "w": W}
        for i in range(N_CORES)
    ]

    nc = _get_nc()
    res = run_bass_kernel_spmd(nc, in_maps, core_ids=list(range(N_CORES)))
    outs = res.results
    ZT = np.concatenate([np.asarray(outs[i]["out"]) for i in range(N_CORES)], axis=1)
    Z = np.ascontiguousarray(ZT[:, :N_NODES].T)  # [N, H]

    A = sp.csr_matrix(
        (edge_vals, (edge_dst.astype(np.int64), edge_src.astype(np.int64))),
        shape=(N_NODES, N_NODES),
    )
    agg = A @ Z
    return np.maximum(agg, 0.0).astype(np.float32)
